# revision 28
# baseline (speedup 1.0000x reference)
"""Trainium2 Bass kernel for DeepSeek-style MLA (multi-head latent attention).

Sharding: 8 cores = 2 (batch) x 4 (head-groups of 4 heads).
Core c handles batch b = c // 4 and heads [4*(c%4), 4*(c%4)+4).
Each core computes its 4 heads' full attention + its partial o_proj
contribution y_partial [S, D]; host sums the 4 partials per batch.

Device layout strategy (per core):
  - All projection outputs are produced transposed (feature dim on
    partitions) so attention needs no on-device transposes of score-sized
    matrices:
      QTn  [128(nope), 4(head), 2048(s)]
      QTp  [128(2x64 pe), 2(head-pair), 2048(s)]  (head h on partitions (h%2)*64..)
      KTn  [128(nope), 4(head), 2048(s)]
      kpeT [64(pe), 2048(s)]                      (shared across heads, RoPE'd)
      Vsb  [128(s within tile), 16(s-tile), 4(head), 128(vd)]
  - Scores computed transposed: ST[k, q] = K @ Q^T; exp on ACT; column
    sums via ones-matmul on PE; PV uses V directly as lhsT; final
    per-column 1/sum via DRAM-bounce partition broadcast.
  - Softmax skips max-subtraction (scores are O(1); exp is safe).
  - RMS-norm weight and the 1/sqrt(192) score scale are folded into the
    weights on the host.

Phases: 0) kv_a + RMS + transposes + k_pe RoPE   1) Q proj + RoPE
        2) kv_b (k_nope^T, V)                    3) attention
        4) o_proj
"""

import math
import sys

import numpy as np

for _p in ("/opt/trn_rl_repo",):
    if _p not in sys.path:
        sys.path.insert(0, _p)

# ---- problem constants (hardcoded per contract) ----
B = 2
S = 2048
D = 2048
H = 16
NOPE = 128
ROPE = 64
VD = 128
KV_RANK = 512
QHD = NOPE + ROPE
EPS = 1e-6
BASE = 10000.0

HPC = 4            # heads per core
NCORES = 8
P = 128
QS = 512           # q-super width
NQ = S // QS       # 4
NST = S // P       # 16 s-tiles
NKC = D // P       # 16 d-chunks
RC = KV_RANK // P  # 4 r-chunks
HALF = ROPE // 2   # 32

USE_F32R = True    # matmul inputs as float32r (single-pass fp32, ~4x faster)

_CACHE = {}


def _emit(nc, tc):
    """Emit the whole per-core program into TileContext tc."""
    import concourse.bass as bass
    import concourse.mybir as mybir

    f32 = mybir.dt.float32
    fr = mybir.dt.float32r if USE_F32R else f32
    AF = mybir.ActivationFunctionType

    def mm(ap):
        return ap

    # ---- DRAM I/O ----
    hT = nc.dram_tensor("hT", [D, S], fr, kind="ExternalInput").ap()
    qwT = nc.dram_tensor("qwT", [D, HPC * QHD], fr, kind="ExternalInput").ap()
    kvawT = nc.dram_tensor("kvawT", [D, KV_RANK + ROPE], fr, kind="ExternalInput").ap()
    kvbk = nc.dram_tensor("kvbk", [P, RC, HPC * NOPE], fr, kind="ExternalInput").ap()
    kvbv = nc.dram_tensor("kvbv", [P, RC, HPC * VD], fr, kind="ExternalInput").ap()
    owT = nc.dram_tensor("owT", [P, HPC, D], fr, kind="ExternalInput").ap()
    cosT2 = nc.dram_tensor("cosT2", [P, S], fr, kind="ExternalInput").ap()
    sinT2 = nc.dram_tensor("sinT2", [P, S], fr, kind="ExternalInput").ap()
    masks = nc.dram_tensor("masks", [P, 4, QS], fr, kind="ExternalInput").ap()
    ones_d = nc.dram_tensor("ones_d", [P, 1], fr, kind="ExternalInput").ap()
    ident = nc.dram_tensor("ident", [P, P], f32, kind="ExternalInput").ap()
    y = nc.dram_tensor("y", [S, D], f32, kind="ExternalOutput").ap()
    # scratch for the [1,512] -> [128,512] partition broadcast (DRAM bounce)
    rsum_d = nc.dram_tensor("rsum_d", [HPC, NQ, QS], f32).ap()
    # DRAM bounces for cross-phase intermediates (SBUF is the scarce resource)
    ckv_d = nc.dram_tensor("ckv_d", [P, RC, S], fr).ap()    # normed ckv^T
    attn_d = nc.dram_tensor("attn_d", [P, HPC, S], fr).ap()  # attn out^T

    # ---- long-lived pools, entered in LIFO-exit order ----
    # (a tile's slot is live from its pool.tile() call to pool exit, so
    # entering early costs nothing)
    p_const_cm = tc.tile_pool(name="const", bufs=1)
    p_const = p_const_cm.__enter__()
    p_qt_cm = tc.tile_pool(name="qt", bufs=1)        # QTn/QTp/kpeT: ph0 -> attn
    p_qt = p_qt_cm.__enter__()
    p_cs_cm = tc.tile_pool(name="cossin", bufs=1)    # cos/sin: ph0 -> ph1
    p_cs = p_cs_cm.__enter__()

    ident_sb = p_const.tile([P, P], f32, name="ident")
    nc.sync.dma_start(out=ident_sb, in_=ident)
    ones_sb = p_const.tile([P, 1], fr, name="ones")
    nc.sync.dma_start(out=ones_sb, in_=ones_d)
    eps_sb = p_const.tile([P, 1], f32, name="eps")
    nc.vector.memset(eps_sb, EPS)
    cos_sb = p_cs.tile([P, S], fr, name="cos")
    sin_sb = p_cs.tile([P, S], fr, name="sin")
    nc.sync.dma_start(out=cos_sb, in_=cosT2)
    nc.sync.dma_start(out=sin_sb, in_=sinT2)

    QTn = p_qt.tile([P, HPC, S], fr, name="QTn")
    QTp = p_qt.tile([P, 2, S], fr, name="QTp")
    # kpeT is duplicated on partitions [0:64] and [64:128] so it can serve
    # as lhsT at either base partition (matmul requires matching bases).
    kpeT = p_qt.tile([P, S], fr, name="kpeT")

    def rope(dst, rot, t1, part_hi, q0, qw):
        """In-place RoPE on dst[:part_hi, ..., q0:q0+qw] (pe-dim on partitions,
        repeating every 64). rot/t1 are scratch tiles of dst-slice shape."""
        dsl = dst[:part_hi, ..., q0:q0 + qw]
        for b0 in range(0, part_hi, ROPE):
            nc.vector.tensor_scalar_mul(
                out=rot[b0:b0 + HALF], in0=dsl[b0 + HALF:b0 + ROPE],
                scalar1=-1.0)
            nc.vector.tensor_copy(
                out=rot[b0 + HALF:b0 + ROPE], in_=dsl[b0:b0 + HALF])
        csl = cos_sb[:part_hi, q0:q0 + qw]
        ssl = sin_sb[:part_hi, q0:q0 + qw]
        if len(rot.shape) == 3:
            csl = cos_sb[:part_hi, None, q0:q0 + qw].to_broadcast(rot.shape)
            ssl = sin_sb[:part_hi, None, q0:q0 + qw].to_broadcast(rot.shape)
        nc.vector.tensor_mul(t1, dsl, csl)
        nc.vector.tensor_mul(rot, rot, ssl)
        nc.vector.tensor_add(dsl, t1, rot)

    # ============ Phase 0: kv_a + RMS + transposes + k_pe RoPE ============
    with tc.tile_pool(name="ph0h", bufs=3) as p_hst, \
         tc.tile_pool(name="ph0c", bufs=1) as ph0c, \
         tc.tile_pool(name="ph0", bufs=2) as ph0, \
         tc.tile_pool(name="ph0r", bufs=1) as ph0r, \
         tc.tile_pool(name="ps0", bufs=2, space="PSUM") as ps0, \
         tc.tile_pool(name="ps0b", bufs=2, space="PSUM") as ps0b:

        kvaw_sb = ph0c.tile([P, NKC, KV_RANK + ROPE], fr, name="kvaw")
        nc.sync.dma_start(out=kvaw_sb, in_=kvawT.rearrange(
            "(kc p) o -> p kc o", p=P))

        for st in range(NST):
            hst = p_hst.tile([P, NKC, P], fr, name="hst")
            nc.sync.dma_start(
                out=hst,
                in_=hT[:, st * P:(st + 1) * P].rearrange(
                    "(kc p) s -> p kc s", p=P))
            ps_ckv = ps0.tile([P, KV_RANK], f32, name="ps_ckv")
            ps_kpe = ps0b.tile([P, ROPE], f32, name="ps_kpe")
            for k in range(NKC):
                nc.tensor.matmul(
                    ps_ckv, mm(hst[:, k, :]), mm(kvaw_sb[:, k, :KV_RANK]),
                    start=(k == 0), stop=(k == NKC - 1))
                nc.tensor.matmul(
                    ps_kpe, mm(hst[:, k, :]), mm(kvaw_sb[:, k, KV_RANK:]),
                    start=(k == 0), stop=(k == NKC - 1))
            # rs = 1/sqrt(mean(ckv^2) + eps)
            ckvn = ph0.tile([P, KV_RANK], f32, name="ckvn")
            ssq = ph0.tile([P, 1], f32, name="ssq")
            nc.scalar.activation(out=ckvn, in_=ps_ckv, func=AF.Square,
                                 accum_out=ssq)
            rs = ph0.tile([P, 1], f32, name="rs")
            nc.scalar.activation(out=rs, in_=ssq, func=AF.Sqrt,
                                 bias=eps_sb, scale=1.0 / KV_RANK)
            nc.vector.reciprocal(rs, rs)
            nc.scalar.activation(out=ckvn, in_=ps_ckv, func=AF.Copy, scale=rs)
            # transpose normed ckv -> ckv^T columns, bounce to DRAM
            ps_tr = ps0b.tile([P, KV_RANK], f32, name="ps_tr")
            for rc in range(RC):
                nc.tensor.transpose(
                    ps_tr[:, rc * P:(rc + 1) * P],
                    ckvn[:, rc * P:(rc + 1) * P], ident_sb)
            cstage = ph0.tile([P, KV_RANK], fr, name="cstage")
            nc.scalar.copy(out=cstage, in_=ps_tr)
            nc.sync.dma_start(
                out=ckv_d[:, :, st * P:(st + 1) * P],
                in_=cstage.rearrange("p (rc s) -> p rc s", rc=RC))
            # k_pe: SBUF-copy, transpose into kpeT columns
            kpes = ph0.tile([P, ROPE], f32, name="kpes")
            nc.scalar.copy(out=kpes, in_=ps_kpe)
            ps_kt = ps0b.tile([ROPE, P], f32, name="ps_kt")
            nc.tensor.transpose(ps_kt, kpes, ident_sb)
            nc.scalar.copy(out=kpeT[:ROPE, st * P:(st + 1) * P], in_=ps_kt)
            nc.scalar.copy(out=kpeT[ROPE:, st * P:(st + 1) * P], in_=ps_kt)
            if st % 4 == 3:  # RoPE the finished quarter of kpeT (both copies)
                rotk = ph0r.tile([P, QS], fr, name="rotk")
                t1k = ph0r.tile([P, QS], fr, name="t1k")
                rope(kpeT, rotk, t1k, P, (st - 3) * P, QS)

    # ============ Phase 1: Q projection + RoPE ============================
    with tc.tile_pool(name="ph1h", bufs=3) as p_htq, \
         tc.tile_pool(name="ph1w", bufs=6) as ph1w, \
         tc.tile_pool(name="ph1r", bufs=1) as ph1r, \
         tc.tile_pool(name="ps1", bufs=2, space="PSUM") as ps1:

        for qtr in range(NQ):
            q0 = qtr * QS
            htq = []
            for kh in range(2):
                t = p_htq.tile([P, NKC // 2, QS], fr, name="htq")
                nc.sync.dma_start(
                    out=t,
                    in_=hT[kh * 1024:(kh + 1) * 1024, q0:q0 + QS].rearrange(
                        "(kk p) s -> p kk s", p=P))
                htq.append(t)

            for m in range(8):
                mw = P if m < 4 else ROPE
                c0 = m * P if m < 4 else 4 * P + (m - 4) * ROPE
                ps_q = ps1.tile([P, QS], f32, name="ps_q")
                for k in range(NKC):
                    wt = ph1w.tile([P, P], fr, name="qw")
                    nc.sync.dma_start(
                        out=wt[:, :mw], in_=qwT[k * P:(k + 1) * P, c0:c0 + mw])
                    nc.tensor.matmul(
                        ps_q[:mw], mm(wt[:, :mw]),
                        mm(htq[k // 8][:, k % 8, :]),
                        start=(k == 0), stop=(k == NKC - 1))
                if m < 4:
                    nc.scalar.copy(out=QTn[:, m, q0:q0 + QS], in_=ps_q)
                else:
                    h = m - 4
                    nc.scalar.copy(
                        out=QTp[(h % 2) * ROPE:(h % 2 + 1) * ROPE,
                                h // 2, q0:q0 + QS],
                        in_=ps_q[:ROPE])
            rot = ph1r.tile([P, 2, QS], fr, name="rot")
            t1 = ph1r.tile([P, 2, QS], fr, name="t1")
            rope(QTp, rot, t1, P, q0, QS)

    p_cs_cm.__exit__(None, None, None)  # free cos/sin

    # ============ Phase 2: kv_b (k_nope^T per head, V) =====================
    p_kvb_cm = tc.tile_pool(name="kvb", bufs=1)      # KTn/Vsb: ph2 -> attn
    p_kvb = p_kvb_cm.__enter__()
    KTn = p_kvb.tile([P, HPC, S], fr, name="KTn")
    Vsb = p_kvb.tile([P, NST, HPC, VD], fr, name="Vsb")

    with tc.tile_pool(name="ph2", bufs=1) as ph2, \
         tc.tile_pool(name="ph2c", bufs=2) as ph2c, \
         tc.tile_pool(name="ps2", bufs=4, space="PSUM") as ps2:
        kvbk_sb = ph2.tile([P, RC, HPC * NOPE], fr, name="kvbk")
        kvbv_sb = ph2.tile([P, RC, HPC * VD], fr, name="kvbv")
        nc.sync.dma_start(out=kvbk_sb, in_=kvbk)
        nc.sync.dma_start(out=kvbv_sb, in_=kvbv)

        for sc in range(NQ):
            cc = ph2c.tile([P, RC, QS], fr, name="cc")
            nc.sync.dma_start(out=cc, in_=ckv_d[:, :, sc * QS:(sc + 1) * QS])
            for h in range(HPC):
                ps = ps2.tile([P, QS], f32, name="ps_kn")
                for rc in range(RC):
                    nc.tensor.matmul(
                        ps, mm(kvbk_sb[:, rc, h * NOPE:(h + 1) * NOPE]),
                        mm(cc[:, rc, :]),
                        start=(rc == 0), stop=(rc == RC - 1))
                nc.scalar.copy(out=KTn[:, h, sc * QS:(sc + 1) * QS], in_=ps)
            for stl in range(4):
                st = sc * 4 + stl
                psv = ps2.tile([P, HPC * VD], f32, name="ps_v")
                for h in range(HPC):
                    for rc in range(RC):
                        nc.tensor.matmul(
                            psv[:, h * VD:(h + 1) * VD],
                            mm(cc[:, rc, stl * P:(stl + 1) * P]),
                            mm(kvbv_sb[:, rc, h * VD:(h + 1) * VD]),
                            start=(rc == 0), stop=(rc == RC - 1))
                nc.scalar.copy(out=Vsb[:, st, :, :],
                               in_=psv.rearrange("p (h v) -> p h v", h=HPC))

    # ============ Phase 3: attention ======================================
    with tc.tile_pool(name="att", bufs=6) as p_att, \
         tc.tile_pool(name="attc", bufs=1) as p_attc, \
         tc.tile_pool(name="attb", bufs=3) as p_attb, \
         tc.tile_pool(name="ps_sc", bufs=3, space="PSUM") as ps_scp, \
         tc.tile_pool(name="ps_sum", bufs=2, space="PSUM") as ps_sump, \
         tc.tile_pool(name="ps_pv", bufs=2, space="PSUM") as ps_pvp:

        mask_sb = p_attc.tile([P, 4, QS], fr, name="masks")
        nc.sync.dma_start(out=mask_sb, in_=masks)

        for h in range(HPC):
            qprhs = QTp[(h % 2) * ROPE:(h % 2 + 1) * ROPE, h // 2, :]
            for qt in range(NQ):
                q0 = qt * QS
                nj = 4 * qt + 4
                ps_sum = ps_sump.tile([1, QS], f32, name="ps_sum")
                ps_pv = ps_pvp.tile([P, QS], f32, name="ps_pv")
                for j in range(nj):
                    ps_sc = ps_scp.tile([P, QS], f32, name="ps_sc")
                    nc.tensor.matmul(
                        ps_sc, mm(KTn[:, h, j * P:(j + 1) * P]),
                        mm(QTn[:, h, q0:q0 + QS]), start=True, stop=False)
                    nc.tensor.matmul(
                        ps_sc,
                        mm(kpeT[(h % 2) * ROPE:(h % 2 + 1) * ROPE,
                                j * P:(j + 1) * P]),
                        mm(qprhs[:, q0:q0 + QS]), start=False, stop=True)
                    ep = p_att.tile([P, QS], fr, name="expP")
                    nc.scalar.activation(out=ep, in_=ps_sc, func=AF.Exp)
                    jl = j - 4 * qt
                    if jl >= 0:  # diagonal super-block: causal mask
                        nc.gpsimd.tensor_mul(ep, ep, mask_sb[:, jl, :])
                    nc.tensor.matmul(
                        ps_sum, mm(ones_sb), mm(ep),
                        start=(j == 0), stop=(j == nj - 1))
                    nc.tensor.matmul(
                        ps_pv, mm(Vsb[:, j, h, :]), mm(ep),
                        start=(j == 0), stop=(j == nj - 1))
                # normalize: attnT[:, h, q0:] = ps_pv * (1/sums)[bcast]
                rsum = p_attb.tile([1, QS], f32, name="rsum")
                nc.vector.reciprocal(rsum, ps_sum)
                nc.sync.dma_start(out=rsum_d[h, qt, :], in_=rsum)
                bcast = p_attb.tile([P, QS], f32, name="bcast")
                src = rsum_d[h, qt, :]
                nc.sync.dma_start(
                    out=bcast,
                    in_=bass.AP(tensor=src.tensor, offset=src.offset,
                                ap=[[0, P]] + list(src.ap)))
                at = p_attb.tile([P, QS], fr, name="at")
                nc.vector.tensor_mul(at, ps_pv, bcast)
                nc.sync.dma_start(out=attn_d[:, h, q0:q0 + QS], in_=at)

    p_kvb_cm.__exit__(None, None, None)
    p_qt_cm.__exit__(None, None, None)

    # ============ Phase 4: o_proj =========================================
    with tc.tile_pool(name="ph4", bufs=1) as ph4, \
         tc.tile_pool(name="ph4a", bufs=3) as ph4a, \
         tc.tile_pool(name="ph4y", bufs=4) as ph4y, \
         tc.tile_pool(name="ps4", bufs=4, space="PSUM") as ps4:
        owT_sb = ph4.tile([P, HPC, D], fr, name="owT")
        nc.sync.dma_start(out=owT_sb, in_=owT)
        for st in range(NST):
            at_t = ph4a.tile([P, HPC, P], fr, name="at_t")
            nc.sync.dma_start(out=at_t,
                              in_=attn_d[:, :, st * P:(st + 1) * P])
            for nk in range(NQ):
                psy = ps4.tile([P, QS], f32, name="psy")
                for hc in range(HPC):
                    nc.tensor.matmul(
                        psy, mm(at_t[:, hc, :]),
                        mm(owT_sb[:, hc, nk * QS:(nk + 1) * QS]),
                        start=(hc == 0), stop=(hc == HPC - 1))
                ys = ph4y.tile([P, QS], f32, name="ys")
                nc.scalar.copy(out=ys, in_=psy)
                nc.sync.dma_start(
                    out=y[st * P:(st + 1) * P, nk * QS:(nk + 1) * QS],
                    in_=ys)

    p_const_cm.__exit__(None, None, None)


def _build_program():
    import concourse.bacc as bacc
    import concourse.tile as tile

    nc = bacc.Bacc("TRN2", target_bir_lowering=False, debug=False,
                   num_devices=NCORES)
    with tile.TileContext(nc) as tc:
        _emit(nc, tc)
    nc.compile()
    return nc


def _rope_cos_sin():
    inv_freq = 1.0 / (BASE ** (np.arange(0, ROPE, 2, dtype=np.float32) / ROPE))
    t = np.arange(S, dtype=np.float32)
    freqs = np.outer(t, inv_freq)                     # [S, ROPE/2]
    emb = np.concatenate([freqs, freqs], axis=-1)     # [S, ROPE]
    return np.cos(emb), np.sin(emb)


def _host_prep(hidden_states, q_proj_w, kv_a_proj_w, kv_a_norm_w,
               kv_b_proj_w, o_proj_w):
    """Build per-core input maps."""
    hidden_states = np.asarray(hidden_states, dtype=np.float32)
    q_proj_w = np.asarray(q_proj_w, dtype=np.float32)
    kv_a_proj_w = np.asarray(kv_a_proj_w, dtype=np.float32)
    kv_a_norm_w = np.asarray(kv_a_norm_w, dtype=np.float32)
    kv_b_proj_w = np.asarray(kv_b_proj_w, dtype=np.float32)
    o_proj_w = np.asarray(o_proj_w, dtype=np.float32)

    scale = np.float32(1.0 / math.sqrt(QHD))
    qws = (q_proj_w * scale).reshape(H, QHD, D)
    kvb = (kv_b_proj_w * kv_a_norm_w[None, :]).reshape(H, NOPE + VD, KV_RANK)

    cos, sin = _rope_cos_sin()                             # [S, ROPE]
    cosT2 = np.ascontiguousarray(np.tile(cos.T, (2, 1)))   # [128, S]
    sinT2 = np.ascontiguousarray(np.tile(sin.T, (2, 1)))

    # diag masks, stored partition-major: masks[p, j, q]
    r = np.arange(P)[:, None]
    ql = np.arange(QS)[None, :]
    masks = np.stack([(ql >= j * P + r).astype(np.float32) for j in range(4)])
    masks = np.ascontiguousarray(masks.transpose(1, 0, 2))  # [128, 4, 512]
    ident = np.eye(P, dtype=np.float32)
    kvawT = np.ascontiguousarray(kv_a_proj_w.T)             # [D, 576]

    in_maps = []
    for c in range(NCORES):
        b, g = divmod(c, HPC)
        heads = list(range(HPC * g, HPC * g + HPC))
        hT = np.ascontiguousarray(hidden_states[b].T)       # [D, S]
        # qwT cols: 4x nope(128) blocks then 4x pe(64) blocks -> [D, 768]
        cols = np.concatenate(
            [qws[h, :NOPE, :] for h in heads]
            + [qws[h, NOPE:, :] for h in heads], axis=0)    # [768, D]
        qwT_c = np.ascontiguousarray(cols.T)
        # kvbk [128, 4, 4*128]: kvbk[p, rc, h*128+j] = kvb[heads[h], j, rc*128+p]
        kn = np.stack([kvb[h, :NOPE, :] for h in heads])    # [h, j, r]
        kvbk_c = np.ascontiguousarray(
            kn.transpose(2, 0, 1).reshape(RC, P, HPC, NOPE)
            .transpose(1, 0, 2, 3).reshape(P, RC, HPC * NOPE))
        kv = np.stack([kvb[h, NOPE:, :] for h in heads])    # [h, j(vd), r]
        kvbv_c = np.ascontiguousarray(
            kv.transpose(2, 0, 1).reshape(RC, P, HPC, VD)
            .transpose(1, 0, 2, 3).reshape(P, RC, HPC * VD))
        # owT [128, 4, D]: owT[p, hc, n] = o_proj_w[n, g*512 + hc*128 + p]
        ow = o_proj_w[:, g * HPC * VD:(g + 1) * HPC * VD]   # [D, 512]
        owT_c = np.ascontiguousarray(
            ow.T.reshape(HPC, VD, D).transpose(1, 0, 2))    # [128, 4, D]
        in_maps.append({
            "hT": hT, "qwT": qwT_c, "kvawT": kvawT,
            "kvbk": kvbk_c, "kvbv": kvbv_c, "owT": owT_c,
            "cosT2": cosT2, "sinT2": sinT2, "masks": masks, "ident": ident,
            "ones_d": np.ones((P, 1), dtype=np.float32),
        })
    return in_maps


def _gather(results):
    out = np.zeros((B, S, D), dtype=np.float32)
    for c in range(NCORES):
        out[c // HPC] += results[c]["y"]
    return out


def kernel(hidden_states, q_proj_w, kv_a_proj_w, kv_a_norm_w,
           kv_b_proj_w, o_proj_w):
    from concourse import bass_utils

    in_maps = _host_prep(hidden_states, q_proj_w, kv_a_proj_w, kv_a_norm_w,
                         kv_b_proj_w, o_proj_w)
    if "nc" not in _CACHE:
        _CACHE["nc"] = _build_program()
    nc = _CACHE["nc"]
    res = bass_utils.run_bass_kernel_spmd(nc, in_maps, list(range(NCORES)))
    return _gather(res.results)


if __name__ == "__main__":
    rng = np.random.default_rng(0)
    ins = {
        "hidden_states": rng.standard_normal((B, S, D), dtype=np.float32),
        "q_proj_w": rng.standard_normal((H * QHD, D), dtype=np.float32) * D ** -0.5,
        "kv_a_proj_w": rng.standard_normal((KV_RANK + ROPE, D), dtype=np.float32) * D ** -0.5,
        "kv_a_norm_w": np.ones(KV_RANK, dtype=np.float32),
        "kv_b_proj_w": rng.standard_normal((H * (NOPE + VD), KV_RANK), dtype=np.float32) * KV_RANK ** -0.5,
        "o_proj_w": rng.standard_normal((D, H * VD), dtype=np.float32) * (H * VD) ** -0.5,
    }
    out = kernel(**ins)
    print(out.shape, out.dtype, float(np.abs(out).mean()))
